# revision 2
# baseline (speedup 1.0000x reference)
"""Trainium2 Bass kernel for the CNN_PHMM_VAE loss — fused fwd/bwd pHMM, v3.

Like kernel_v2 (fused forward+backward halves, 528-col flat bf16 ops, host
finish), but ALL compute on the Vector engine: GpSimd shares an SBUF port
with DVE and concurrent GpSimd ops were measured to slow DVE ops up to 5x,
which cost more than GpSimd contributed. Also: single TAB stream (the
delete-scan input is rebuilt from S' with a static Q tile), and rescales use
per-partition power-of-2 scales (tensor_scalar 4x mode) — safe because
after host centering the inter-group drift within a partition is << bf16
range.
"""

import sys

import numpy as np

if "/opt/trn_rl_repo" not in sys.path:
    sys.path.insert(0, "/opt/trn_rl_repo")

import ml_dtypes

BF16 = np.dtype(ml_dtypes.bfloat16)

B, K, L, E = 4096, 64, 128, 16
NCORES = 8
BPC = B // NCORES
G = BPC // 128
GS = 66
GB = 2 * GS
F = G * GB                  # 528
N = L // 2
RESCALE_AT = tuple(int(v) for v in __import__('os').environ.get('RESCALE_AT','22,44').split(',') if v)

M2M, M2I, M2D, I2M, I2I, D2M, D2D = range(7)

_CACHE = {}


def _build_program():
    import concourse.bacc as bacc
    import concourse.mybir as mybir
    from concourse.tile import TileContext

    bf16 = mybir.dt.bfloat16
    f32 = mybir.dt.float32
    i32 = mybir.dt.int32
    MULT = mybir.AluOpType.mult
    ADD = mybir.AluOpType.add
    MAX = mybir.AluOpType.max
    SUB = mybir.AluOpType.subtract
    LSR = mybir.AluOpType.logical_shift_right
    AND = mybir.AluOpType.bitwise_and
    X = mybir.AxisListType.X

    nc = bacc.Bacc("TRN2", target_bir_lowering=False, debug=False,
                   num_devices=NCORES)

    tab_h = nc.declare_dram_parameter("tab", [N, 128, F], bf16, isOutput=False)
    c12_h = nc.declare_dram_parameter("c12", [128, 2 * F], bf16, isOutput=False)
    atil_h = nc.declare_dram_parameter("atil", [128, F], bf16, isOutput=False)
    qcol_h = nc.declare_dram_parameter("qcol", [128, F], bf16, isOutput=False)
    big0_h = nc.declare_dram_parameter("big0", [128, 2 * F], bf16, isOutput=False)
    p0_h = nc.declare_dram_parameter("p0", [128, F + 2], bf16, isOutput=False)
    cinit_h = nc.declare_dram_parameter("cinit", [128, 8], f32, isOutput=False)
    pout_h = nc.declare_dram_parameter("pout", [128, F], f32, isOutput=True)
    sgout_h = nc.declare_dram_parameter("sgout", [128, 2 * F], f32, isOutput=True)
    cacc_h = nc.declare_dram_parameter("cacc", [128, 8], f32, isOutput=True)

    with TileContext(nc) as tc:
        with tc.tile_pool(name="consts", bufs=1) as consts, \
             tc.tile_pool(name="state", bufs=1) as state, \
             tc.tile_pool(name="tmps", bufs=2) as tmps, \
             tc.tile_pool(name="stream", bufs=8) as stream:

            C12 = consts.tile([128, 2 * F], bf16)
            nc.sync.dma_start(C12[:], c12_h[:])
            ATIL = consts.tile([128, F], bf16)
            nc.sync.dma_start(ATIL[:], atil_h[:])
            QCOL = consts.tile([128, F], bf16)
            nc.sync.dma_start(QCOL[:], qcol_h[:])

            # [pad2 | S_e | GIa_e | S_o | GIa_o]
            BIGT = state.tile([128, 2 + 4 * F], bf16)
            nc.vector.memset(BIGT[:, 0:2], 0.0)
            nc.sync.dma_start(BIGT[:, 2:2 + 2 * F], big0_h[:])
            nc.vector.memset(BIGT[:, 2 + 2 * F:2 + 4 * F], 0.0)
            PT = [state.tile([128, F + 2], bf16, name=f"P{j}") for j in range(2)]
            nc.sync.dma_start(PT[0][:], p0_h[:])
            nc.vector.memset(PT[1][:], 0.0)
            CACC = state.tile([128, 8], f32)
            nc.sync.dma_start(CACC[:], cinit_h[:])

            def s_off(h):
                return 2 + 2 * F * h

            def s_half(h):
                o = s_off(h)
                return BIGT[:, o:o + F]

            def gia_half(h):
                o = s_off(h) + F
                return BIGT[:, o:o + F]

            def pair_half(h):
                o = s_off(h)
                return BIGT[:, o:o + 2 * F]

            def s_shift_half(h):
                o = s_off(h)
                return BIGT[:, o - 1:o - 1 + F]

            tpair = tmps.tile([128, 2 * F], bf16, name="tp", tag="tp")
            nc.vector.tensor_tensor(tpair[:], C12[:], pair_half(0), MULT)

            for i in range(1, N + 1):
                prev_h, cur_h = (i + 1) % 2, i % 2
                Pprev, Pcur = PT[prev_h], PT[cur_h]
                tab = stream.tile([128, F], bf16, name="tab", tag="tab")
                nc.sync.dma_start(tab[:], tab_h[i - 1])

                # S' = TAB * P[-1]   (tab col0 = 0 -> writes S'[0] = 0)
                nc.vector.tensor_tensor(s_half(cur_h), tab[:],
                                        Pprev[:, 1:F + 1], MULT)
                # qs = Q * S'[-1]
                qs = tmps.tile([128, F], bf16, name="qs", tag="qs")
                nc.vector.tensor_tensor(qs[:], QCOL[:], s_shift_half(cur_h),
                                        MULT)
                # dd = scan(ATIL, qs)
                dd = tmps.tile([128, F], bf16, name="dd", tag="dd")
                nc.vector.tensor_tensor_scan(dd[:], ATIL[:], qs[:],
                                             0.0, MULT, ADD)
                # GIa = t1 + t2 ; h = S' + GIa ; P = h + dd
                nc.vector.tensor_tensor(gia_half(cur_h), tpair[:, 0:F],
                                        tpair[:, F:2 * F], ADD)
                hh = tmps.tile([128, F], bf16, name="hh", tag="hh")
                nc.vector.tensor_tensor(hh[:], s_half(cur_h), gia_half(cur_h),
                                        ADD)
                nc.vector.tensor_tensor(Pcur[:, 2:F + 2], hh[:], dd[:], ADD)

                if i in RESCALE_AT:
                    # power-of-2 rescale per (partition, group, half): one
                    # scale per row-half, broadcast over its 66 columns
                    P3 = Pcur[:, 2:F + 2].rearrange("p (g k) -> p g k", g=8)
                    rm = tmps.tile([128, 8], f32, name="rm", tag="rm")
                    nc.vector.tensor_reduce(rm[:], P3, X, MAX)
                    nc.vector.tensor_scalar_max(rm[:], rm[:], 1e-30)
                    mask = tmps.tile([128, 8], i32, name="msk", tag="msk")
                    nc.vector.tensor_scalar(mask[:], rm.bitcast(i32),
                                            0x7F800000, None, AND)
                    rib = tmps.tile([128, 8], i32, name="rib", tag="rib")
                    nc.vector.tensor_scalar(rib[:], mask[:], -1, 0x7F000000,
                                            MULT, ADD)
                    rinv = tmps.tile([128, 8], f32, name="riv", tag="riv")
                    nc.vector.tensor_copy(rinv.bitcast(i32), rib[:])
                    es = tmps.tile([128, 8], i32, name="es", tag="es")
                    nc.vector.tensor_scalar(es[:], mask[:], 23, None, LSR)
                    ef = tmps.tile([128, 8], f32, name="ef", tag="ef")
                    nc.vector.tensor_copy(ef[:], es[:])
                    el = tmps.tile([128, 8], f32, name="el", tag="el")
                    nc.vector.tensor_scalar(el[:], ef[:], 127.0,
                                            float(np.log(2.0)), SUB, MULT)
                    nc.vector.tensor_tensor(CACC[:], CACC[:], el[:], ADD)
                    rb = tmps.tile([128, 8], bf16, name="rb", tag="rb")
                    nc.vector.tensor_copy(rb[:], rinv[:])
                    sc3 = rb[:, :, None].to_broadcast((128, 8, GS))
                    nc.vector.tensor_tensor(P3, P3, sc3, MULT)
                    o = s_off(cur_h)
                    S3 = BIGT[:, o:o + F].rearrange("p (g k) -> p g k", g=8)
                    G3 = BIGT[:, o + F:o + 2 * F].rearrange(
                        "p (g k) -> p g k", g=8)
                    nc.vector.tensor_tensor(S3, S3, sc3, MULT)
                    nc.vector.tensor_tensor(G3, G3, sc3, MULT)

                # t-pair for step i+1
                tpair = tmps.tile([128, 2 * F], bf16, name="tp", tag="tp")
                nc.vector.tensor_tensor(tpair[:], C12[:], pair_half(cur_h),
                                        MULT)

            fin_h = N % 2
            po = consts.tile([128, F], f32)
            nc.vector.tensor_copy(po[:], PT[fin_h][:, 2:F + 2])
            nc.sync.dma_start(pout_h[:], po[:])
            sg = consts.tile([128, 2 * F], f32)
            nc.vector.tensor_copy(sg[:], pair_half(fin_h))
            nc.sync.dma_start(sgout_h[:], sg[:])
            nc.sync.dma_start(cacc_h[:], CACC[:])

    nc.compile()
    return nc


def _to_pg(arr):
    tail = arr.shape[1:]
    return arr.reshape(NCORES, G, 128, *tail).transpose(
        0, 2, 1, *range(3, 3 + len(tail)))


def _host_prep(batch_input, transition_probs, emission_probs):
    x = np.asarray(batch_input)
    a = np.asarray(transition_probs, np.float64)
    e = np.asarray(emission_probs, np.float64)

    aM2M, aM2I, aM2D = a[:, :, M2M], a[:, :, M2I], a[:, :, M2D]
    aI2M, aI2I = a[:, :, I2M], a[:, :, I2I]
    aD2M, aD2D = a[:, :, D2M], a[:, :, D2D]

    mu = aM2M[:, 1:].mean(axis=1) + e.mean(axis=(1, 2))

    C1 = 0.25 * np.exp(aI2M + aM2I - aM2M - mu[:, None])
    C2 = 0.25 * np.exp(aI2I - mu[:, None])
    Qf = np.zeros((B, 65))
    Qf[:, 1:] = np.exp(aD2M[:, 1:] + aM2D[:, :-1] - aM2M[:, :-1])
    Af = np.zeros((B, 65))
    Af[:, 1:] = np.exp(aD2M[:, 1:] + aD2D[:, :-1] - aD2M[:, :-1])
    Qb = np.zeros((B, 65))
    Qb[:, 0:64] = np.exp(aM2D[:, 0:64] + aD2M[:, 1:65] - aM2M[:, 1:65])
    Ab = np.zeros((B, 65))
    Ab[:, 0:64] = np.exp(aM2D[:, 0:64] + aD2D[:, 1:65] - aM2D[:, 1:65])

    # layout: per row a fwd 66-col block and a bwd 66-col block; on device the
    # four groups' fwd blocks are contiguous (cols 0:264) then the bwd blocks
    def to_col(fwd65, bwd65_by_k):
        f = np.zeros((B, GS))
        f[:, 0:65] = fwd65
        bwd = np.zeros((B, GS))
        bwd[:, 0:65] = bwd65_by_k[:, ::-1]
        return f, bwd

    C1col = to_col(C1, C1)
    C2col = to_col(C2, C2)
    Qcol = to_col(Qf, Qb)
    Acol = to_col(Af, Ab)

    TABf = np.zeros((N, B, GS), np.float32)
    TABb = np.zeros((N, B, GS), np.float32)
    bidx = np.arange(B)[:, None]
    kf = np.arange(64)[None, :]
    kk = (64 - np.arange(1, 65))[None, :]
    for i in range(1, N + 1):
        TABf[i - 1, :, 1:65] = np.exp(
            aM2M[:, 1:65] + e[bidx, kf, x[:, i - 1][:, None]] - mu[:, None])
        TABb[i - 1, :, 1:65] = np.exp(
            aM2M[:, kk[0]] + e[bidx, kk, x[:, L - i][:, None]] - mu[:, None])

    fD0 = np.full((B, 65), -np.inf)
    fD0[:, 1] = aM2D[:, 0]
    fD0[:, 2:] = aM2D[:, 0:1] + np.cumsum(aD2D[:, 1:64], axis=1)
    gm0 = np.full((B, 65), -np.inf)
    gm0[:, 0] = aM2M[:, 0]
    gd0 = fD0 + aD2M
    gd0[:, 0] = -np.inf
    c0f = np.maximum(gm0.max(axis=1), gd0.max(axis=1))
    GM0 = np.exp(gm0 - c0f[:, None])
    GD0 = np.exp(gd0 - c0f[:, None])
    lnbD = np.empty((B, 65))
    lnbD[:, 64] = aD2M[:, 64]
    rev_cs = np.cumsum(aD2D[:, ::-1][:, 1:65], axis=1)[:, ::-1]
    lnbD[:, 0:64] = rev_cs + aD2M[:, 64:65]
    lnbM = np.empty((B, 65))
    lnbM[:, 64] = aM2M[:, 64]
    lnbM[:, 0:64] = aM2D[:, 0:64] + lnbD[:, 1:65]
    c0b = lnbM.max(axis=1)
    BP0 = np.exp(lnbM - c0b[:, None])
    SIa0 = np.zeros((B, 65))
    SIa0[:, 64] = np.exp(aM2I[:, 64] + aI2M[:, 64] - aI2I[:, 64] - c0b)

    P0 = (np.pad(GM0 + GD0, ((0, 0), (0, 1))),
          np.pad(BP0[:, ::-1], ((0, 0), (0, 1))))
    S0 = (np.pad(GM0, ((0, 0), (0, 1))), np.zeros((B, GS)))
    GIa0 = (np.zeros((B, GS)), np.pad(SIa0[:, ::-1], ((0, 0), (0, 1))))

    w1 = np.exp(-aM2M)
    w2 = np.exp(mu[:, None] - (aI2M + aM2I)) / 0.25

    def pack(fb):
        f, bwd = fb
        return np.concatenate(
            [_to_pg(f).reshape(NCORES, 128, F // 2),
             _to_pg(bwd).reshape(NCORES, 128, F // 2)], axis=2)

    c12 = np.concatenate([pack(C1col), pack(C2col)], axis=2)
    atil = pack(Acol)
    qcol = pack(Qcol)
    big0 = np.concatenate([pack(S0), pack(GIa0)], axis=2)
    p0 = pack((P0[0], P0[1]))
    p0 = np.concatenate([np.zeros((NCORES, 128, 2)), p0], axis=2)
    # per-partition CACC init: groups of one partition share the scale terms
    # only through the host finish, which reads per-row c0 separately; device
    # CACC tracks only the (shared) rescale exponents, init 0.
    cinit = np.zeros((NCORES, 128, 8), np.float32)
    tab = np.concatenate([
        TABf.reshape(N, NCORES, G, 128, GS).transpose(1, 0, 3, 2, 4)
            .reshape(NCORES, N, 128, F // 2),
        TABb.reshape(N, NCORES, G, 128, GS).transpose(1, 0, 3, 2, 4)
            .reshape(NCORES, N, 128, F // 2)], axis=3)

    in_maps = []
    for c in range(NCORES):
        in_maps.append({
            "tab": np.ascontiguousarray(tab[c]).astype(BF16),
            "c12": np.ascontiguousarray(c12[c]).astype(BF16),
            "atil": np.ascontiguousarray(atil[c]).astype(BF16),
            "qcol": np.ascontiguousarray(qcol[c]).astype(BF16),
            "big0": np.ascontiguousarray(big0[c]).astype(BF16),
            "p0": np.ascontiguousarray(p0[c]).astype(BF16),
            "cinit": np.ascontiguousarray(cinit[c]),
        })
    host = dict(w1=w1, w2=w2, C1=C1, C2=C2, mu=mu,
                c0=np.stack([c0f, c0b], axis=1))
    return in_maps, host


def _host_finish(res, host, mus, logvars):
    w1, w2 = host["w1"], host["w2"]
    C1, C2, mu, c0 = host["C1"], host["C2"], host["mu"], host["c0"]

    SG = np.stack([np.asarray(res.results[c]["sgout"], np.float32)
                   for c in range(NCORES)])
    PO = np.stack([np.asarray(res.results[c]["pout"], np.float32)
                   for c in range(NCORES)])
    CA = np.stack([np.asarray(res.results[c]["cacc"], np.float32)
                   for c in range(NCORES)])                 # [NC,128,2]

    H = F // 2

    def rows(arr, lo):  # [NC,128,F-ish] half-slice -> [B,GS]
        return arr[:, :, lo:lo + H].reshape(NCORES, 128, G, GS) \
            .transpose(0, 2, 1, 3).reshape(B, GS)

    Pf = rows(PO, 0).astype(np.float64)
    Pb = rows(PO, H).astype(np.float64)
    Sf = rows(SG, 0).astype(np.float64)
    Sb = rows(SG, H).astype(np.float64)
    Gf = rows(SG[:, :, F:2 * F], 0).astype(np.float64)
    Gb = rows(SG[:, :, F:2 * F], H).astype(np.float64)
    # device CACC: [NC,128,8] = fwd g0..g3, bwd g0..g3 per partition
    cf_rows = CA[:, :, 0:4].transpose(0, 2, 1).reshape(B)
    cb_rows = CA[:, :, 4:8].transpose(0, 2, 1).reshape(B)

    GM = Sf[:, 0:65]
    GI = Gf[:, 0:65]
    bM = Pb[:, 0:65][:, ::-1]
    SIx = (C1 * Sb[:, 0:65][:, ::-1] + C2 * Gb[:, 0:65][:, ::-1])
    tot = (GM * bM * w1 + GI * SIx * w2).sum(axis=1)
    lnP = np.log(np.maximum(tot, 1e-300)) + c0[:, 0] + c0[:, 1] \
        + cf_rows + cb_rows + L * mu
    recon = float(np.mean(-lnP))

    mus = np.asarray(mus, np.float64)
    lv = np.asarray(logvars, np.float64)
    kld = float(np.mean(-0.5 * np.sum(1.0 + lv - mus * mus - np.exp(lv),
                                      axis=1)))
    return np.float32(recon + kld)


def kernel(batch_input, transition_probs, emission_probs, mus, logvars,
           _trace=False, _trace_kwargs=None):
    from concourse.bass_utils import run_bass_kernel_spmd

    if "nc" not in _CACHE:
        _CACHE["nc"] = _build_program()
    nc = _CACHE["nc"]

    in_maps, host = _host_prep(batch_input, transition_probs, emission_probs)
    kw = {}
    if _trace:
        kw["trace"] = True
        kw.update(_trace_kwargs or {})
    res = run_bass_kernel_spmd(nc, in_maps, list(range(NCORES)), **kw)
    _CACHE["last_results"] = res

    return _host_finish(res, host, mus, logvars)


# revision 3
# speedup vs baseline: 1.0694x; 1.0694x over previous
"""Trainium2 Bass kernel for the CNN_PHMM_VAE loss — fused fwd/bwd pHMM, v3.

Like kernel_v2 (fused forward+backward halves, 528-col flat bf16 ops, host
finish), but ALL compute on the Vector engine: GpSimd shares an SBUF port
with DVE and concurrent GpSimd ops were measured to slow DVE ops up to 5x,
which cost more than GpSimd contributed. Also: single TAB stream (the
delete-scan input is rebuilt from S' with a static Q tile), and rescales use
per-partition power-of-2 scales (tensor_scalar 4x mode) — safe because
after host centering the inter-group drift within a partition is << bf16
range.
"""

import sys

import numpy as np

if "/opt/trn_rl_repo" not in sys.path:
    sys.path.insert(0, "/opt/trn_rl_repo")

import ml_dtypes

BF16 = np.dtype(ml_dtypes.bfloat16)

B, K, L, E = 4096, 64, 128, 16
NCORES = 8
BPC = B // NCORES
G = BPC // 128
GS = 66
GB = 2 * GS
F = G * GB                  # 528
N = L // 2
RESCALE_AT = (22, 44)

M2M, M2I, M2D, I2M, I2I, D2M, D2D = range(7)

_CACHE = {}


def _build_program():
    import concourse.bacc as bacc
    import concourse.mybir as mybir
    from concourse.tile import TileContext

    bf16 = mybir.dt.bfloat16
    f32 = mybir.dt.float32
    i32 = mybir.dt.int32
    MULT = mybir.AluOpType.mult
    ADD = mybir.AluOpType.add
    MAX = mybir.AluOpType.max
    SUB = mybir.AluOpType.subtract
    LSR = mybir.AluOpType.logical_shift_right
    AND = mybir.AluOpType.bitwise_and
    X = mybir.AxisListType.X

    nc = bacc.Bacc("TRN2", target_bir_lowering=False, debug=False,
                   num_devices=NCORES)

    tab_h = nc.declare_dram_parameter("tab", [N, 128, F], bf16, isOutput=False)
    c12_h = nc.declare_dram_parameter("c12", [128, 2 * F], bf16, isOutput=False)
    atil_h = nc.declare_dram_parameter("atil", [128, F], bf16, isOutput=False)
    qcol_h = nc.declare_dram_parameter("qcol", [128, F], bf16, isOutput=False)
    big0_h = nc.declare_dram_parameter("big0", [128, 2 * F], bf16, isOutput=False)
    p0_h = nc.declare_dram_parameter("p0", [128, F + 2], bf16, isOutput=False)
    cinit_h = nc.declare_dram_parameter("cinit", [128, 8], f32, isOutput=False)
    pout_h = nc.declare_dram_parameter("pout", [128, F], f32, isOutput=True)
    sgout_h = nc.declare_dram_parameter("sgout", [128, 2 * F], f32, isOutput=True)
    cacc_h = nc.declare_dram_parameter("cacc", [128, 8], f32, isOutput=True)

    with TileContext(nc) as tc:
        with tc.tile_pool(name="consts", bufs=1) as consts, \
             tc.tile_pool(name="state", bufs=1) as state, \
             tc.tile_pool(name="tmps", bufs=2) as tmps, \
             tc.tile_pool(name="stream", bufs=8) as stream:

            C12 = consts.tile([128, 2 * F], bf16)
            nc.sync.dma_start(C12[:], c12_h[:])
            ATIL = consts.tile([128, F], bf16)
            nc.sync.dma_start(ATIL[:], atil_h[:])
            QCOL = consts.tile([128, F], bf16)
            nc.sync.dma_start(QCOL[:], qcol_h[:])

            # [pad2 | S_e | GIa_e | S_o | GIa_o]
            BIGT = state.tile([128, 2 + 4 * F], bf16)
            nc.vector.memset(BIGT[:, 0:2], 0.0)
            nc.sync.dma_start(BIGT[:, 2:2 + 2 * F], big0_h[:])
            nc.vector.memset(BIGT[:, 2 + 2 * F:2 + 4 * F], 0.0)
            PT = [state.tile([128, F + 2], bf16, name=f"P{j}") for j in range(2)]
            nc.sync.dma_start(PT[0][:], p0_h[:])
            nc.vector.memset(PT[1][:], 0.0)
            CACC = state.tile([128, 8], f32)
            nc.sync.dma_start(CACC[:], cinit_h[:])

            def s_off(h):
                return 2 + 2 * F * h

            def s_half(h):
                o = s_off(h)
                return BIGT[:, o:o + F]

            def gia_half(h):
                o = s_off(h) + F
                return BIGT[:, o:o + F]

            def pair_half(h):
                o = s_off(h)
                return BIGT[:, o:o + 2 * F]

            def s_shift_half(h):
                o = s_off(h)
                return BIGT[:, o - 1:o - 1 + F]

            tpair = tmps.tile([128, 2 * F], bf16, name="tp", tag="tp")
            nc.vector.tensor_tensor(tpair[:], C12[:], pair_half(0), MULT)

            for i in range(1, N + 1):
                prev_h, cur_h = (i + 1) % 2, i % 2
                Pprev, Pcur = PT[prev_h], PT[cur_h]
                tab = stream.tile([128, F], bf16, name="tab", tag="tab")
                nc.sync.dma_start(tab[:], tab_h[i - 1])

                # S' = TAB * P[-1]   (tab col0 = 0 -> writes S'[0] = 0)
                nc.vector.tensor_tensor(s_half(cur_h), tab[:],
                                        Pprev[:, 1:F + 1], MULT)
                # qs = Q * S'[-1]
                qs = tmps.tile([128, F], bf16, name="qs", tag="qs")
                nc.vector.tensor_tensor(qs[:], QCOL[:], s_shift_half(cur_h),
                                        MULT)
                # dd = scan(ATIL, qs)
                dd = tmps.tile([128, F], bf16, name="dd", tag="dd")
                nc.vector.tensor_tensor_scan(dd[:], ATIL[:], qs[:],
                                             0.0, MULT, ADD)
                # GIa = t1 + t2 ; h = S' + GIa ; P = h + dd
                nc.vector.tensor_tensor(gia_half(cur_h), tpair[:, 0:F],
                                        tpair[:, F:2 * F], ADD)
                hh = tmps.tile([128, F], bf16, name="hh", tag="hh")
                nc.vector.tensor_tensor(hh[:], s_half(cur_h), gia_half(cur_h),
                                        ADD)
                nc.vector.tensor_tensor(Pcur[:, 2:F + 2], hh[:], dd[:], ADD)

                if i in RESCALE_AT:
                    # power-of-2 rescale per (partition, group, half): one
                    # scale per row-half, broadcast over its 66 columns
                    P3 = Pcur[:, 2:F + 2].rearrange("p (g k) -> p g k", g=8)
                    rm = tmps.tile([128, 8], f32, name="rm", tag="rm")
                    nc.vector.tensor_reduce(rm[:], P3, X, MAX)
                    nc.vector.tensor_scalar_max(rm[:], rm[:], 1e-30)
                    mask = tmps.tile([128, 8], i32, name="msk", tag="msk")
                    nc.vector.tensor_scalar(mask[:], rm.bitcast(i32),
                                            0x7F800000, None, AND)
                    rib = tmps.tile([128, 8], i32, name="rib", tag="rib")
                    nc.vector.tensor_scalar(rib[:], mask[:], -1, 0x7F000000,
                                            MULT, ADD)
                    rinv = tmps.tile([128, 8], f32, name="riv", tag="riv")
                    nc.vector.tensor_copy(rinv.bitcast(i32), rib[:])
                    es = tmps.tile([128, 8], i32, name="es", tag="es")
                    nc.vector.tensor_scalar(es[:], mask[:], 23, None, LSR)
                    ef = tmps.tile([128, 8], f32, name="ef", tag="ef")
                    nc.vector.tensor_copy(ef[:], es[:])
                    el = tmps.tile([128, 8], f32, name="el", tag="el")
                    nc.vector.tensor_scalar(el[:], ef[:], 127.0,
                                            float(np.log(2.0)), SUB, MULT)
                    nc.vector.tensor_tensor(CACC[:], CACC[:], el[:], ADD)
                    rb = tmps.tile([128, 8], bf16, name="rb", tag="rb")
                    nc.vector.tensor_copy(rb[:], rinv[:])
                    sc3 = rb[:, :, None].to_broadcast((128, 8, GS))
                    nc.vector.tensor_tensor(P3, P3, sc3, MULT)
                    o = s_off(cur_h)
                    S3 = BIGT[:, o:o + F].rearrange("p (g k) -> p g k", g=8)
                    G3 = BIGT[:, o + F:o + 2 * F].rearrange(
                        "p (g k) -> p g k", g=8)
                    nc.vector.tensor_tensor(S3, S3, sc3, MULT)
                    nc.vector.tensor_tensor(G3, G3, sc3, MULT)

                # t-pair for step i+1
                tpair = tmps.tile([128, 2 * F], bf16, name="tp", tag="tp")
                nc.vector.tensor_tensor(tpair[:], C12[:], pair_half(cur_h),
                                        MULT)

            fin_h = N % 2
            po = consts.tile([128, F], f32)
            nc.vector.tensor_copy(po[:], PT[fin_h][:, 2:F + 2])
            nc.sync.dma_start(pout_h[:], po[:])
            sg = consts.tile([128, 2 * F], f32)
            nc.vector.tensor_copy(sg[:], pair_half(fin_h))
            nc.sync.dma_start(sgout_h[:], sg[:])
            nc.sync.dma_start(cacc_h[:], CACC[:])

    nc.compile()
    return nc


def _to_pg(arr):
    tail = arr.shape[1:]
    return arr.reshape(NCORES, G, 128, *tail).transpose(
        0, 2, 1, *range(3, 3 + len(tail)))


def _host_prep(batch_input, transition_probs, emission_probs):
    x = np.asarray(batch_input)
    a = np.asarray(transition_probs, np.float64)
    e = np.asarray(emission_probs, np.float64)

    aM2M, aM2I, aM2D = a[:, :, M2M], a[:, :, M2I], a[:, :, M2D]
    aI2M, aI2I = a[:, :, I2M], a[:, :, I2I]
    aD2M, aD2D = a[:, :, D2M], a[:, :, D2D]

    mu = aM2M[:, 1:].mean(axis=1) + e.mean(axis=(1, 2))

    C1 = 0.25 * np.exp(aI2M + aM2I - aM2M - mu[:, None])
    C2 = 0.25 * np.exp(aI2I - mu[:, None])
    Qf = np.zeros((B, 65))
    Qf[:, 1:] = np.exp(aD2M[:, 1:] + aM2D[:, :-1] - aM2M[:, :-1])
    Af = np.zeros((B, 65))
    Af[:, 1:] = np.exp(aD2M[:, 1:] + aD2D[:, :-1] - aD2M[:, :-1])
    Qb = np.zeros((B, 65))
    Qb[:, 0:64] = np.exp(aM2D[:, 0:64] + aD2M[:, 1:65] - aM2M[:, 1:65])
    Ab = np.zeros((B, 65))
    Ab[:, 0:64] = np.exp(aM2D[:, 0:64] + aD2D[:, 1:65] - aM2D[:, 1:65])

    # layout: per row a fwd 66-col block and a bwd 66-col block; on device the
    # four groups' fwd blocks are contiguous (cols 0:264) then the bwd blocks
    def to_col(fwd65, bwd65_by_k):
        f = np.zeros((B, GS))
        f[:, 0:65] = fwd65
        bwd = np.zeros((B, GS))
        bwd[:, 0:65] = bwd65_by_k[:, ::-1]
        return f, bwd

    C1col = to_col(C1, C1)
    C2col = to_col(C2, C2)
    Qcol = to_col(Qf, Qb)
    Acol = to_col(Af, Ab)

    TABf = np.zeros((N, B, GS), np.float32)
    TABb = np.zeros((N, B, GS), np.float32)
    bidx = np.arange(B)[:, None]
    kf = np.arange(64)[None, :]
    kk = (64 - np.arange(1, 65))[None, :]
    for i in range(1, N + 1):
        TABf[i - 1, :, 1:65] = np.exp(
            aM2M[:, 1:65] + e[bidx, kf, x[:, i - 1][:, None]] - mu[:, None])
        TABb[i - 1, :, 1:65] = np.exp(
            aM2M[:, kk[0]] + e[bidx, kk, x[:, L - i][:, None]] - mu[:, None])

    fD0 = np.full((B, 65), -np.inf)
    fD0[:, 1] = aM2D[:, 0]
    fD0[:, 2:] = aM2D[:, 0:1] + np.cumsum(aD2D[:, 1:64], axis=1)
    gm0 = np.full((B, 65), -np.inf)
    gm0[:, 0] = aM2M[:, 0]
    gd0 = fD0 + aD2M
    gd0[:, 0] = -np.inf
    c0f = np.maximum(gm0.max(axis=1), gd0.max(axis=1))
    GM0 = np.exp(gm0 - c0f[:, None])
    GD0 = np.exp(gd0 - c0f[:, None])
    lnbD = np.empty((B, 65))
    lnbD[:, 64] = aD2M[:, 64]
    rev_cs = np.cumsum(aD2D[:, ::-1][:, 1:65], axis=1)[:, ::-1]
    lnbD[:, 0:64] = rev_cs + aD2M[:, 64:65]
    lnbM = np.empty((B, 65))
    lnbM[:, 64] = aM2M[:, 64]
    lnbM[:, 0:64] = aM2D[:, 0:64] + lnbD[:, 1:65]
    c0b = lnbM.max(axis=1)
    BP0 = np.exp(lnbM - c0b[:, None])
    SIa0 = np.zeros((B, 65))
    SIa0[:, 64] = np.exp(aM2I[:, 64] + aI2M[:, 64] - aI2I[:, 64] - c0b)

    P0 = (np.pad(GM0 + GD0, ((0, 0), (0, 1))),
          np.pad(BP0[:, ::-1], ((0, 0), (0, 1))))
    S0 = (np.pad(GM0, ((0, 0), (0, 1))), np.zeros((B, GS)))
    GIa0 = (np.zeros((B, GS)), np.pad(SIa0[:, ::-1], ((0, 0), (0, 1))))

    w1 = np.exp(-aM2M)
    w2 = np.exp(mu[:, None] - (aI2M + aM2I)) / 0.25

    def pack(fb):
        f, bwd = fb
        return np.concatenate(
            [_to_pg(f).reshape(NCORES, 128, F // 2),
             _to_pg(bwd).reshape(NCORES, 128, F // 2)], axis=2)

    c12 = np.concatenate([pack(C1col), pack(C2col)], axis=2)
    atil = pack(Acol)
    qcol = pack(Qcol)
    big0 = np.concatenate([pack(S0), pack(GIa0)], axis=2)
    p0 = pack((P0[0], P0[1]))
    p0 = np.concatenate([np.zeros((NCORES, 128, 2)), p0], axis=2)
    # per-partition CACC init: groups of one partition share the scale terms
    # only through the host finish, which reads per-row c0 separately; device
    # CACC tracks only the (shared) rescale exponents, init 0.
    cinit = np.zeros((NCORES, 128, 8), np.float32)
    tab = np.concatenate([
        TABf.reshape(N, NCORES, G, 128, GS).transpose(1, 0, 3, 2, 4)
            .reshape(NCORES, N, 128, F // 2),
        TABb.reshape(N, NCORES, G, 128, GS).transpose(1, 0, 3, 2, 4)
            .reshape(NCORES, N, 128, F // 2)], axis=3)

    in_maps = []
    for c in range(NCORES):
        in_maps.append({
            "tab": np.ascontiguousarray(tab[c]).astype(BF16),
            "c12": np.ascontiguousarray(c12[c]).astype(BF16),
            "atil": np.ascontiguousarray(atil[c]).astype(BF16),
            "qcol": np.ascontiguousarray(qcol[c]).astype(BF16),
            "big0": np.ascontiguousarray(big0[c]).astype(BF16),
            "p0": np.ascontiguousarray(p0[c]).astype(BF16),
            "cinit": np.ascontiguousarray(cinit[c]),
        })
    host = dict(w1=w1, w2=w2, C1=C1, C2=C2, mu=mu,
                c0=np.stack([c0f, c0b], axis=1))
    return in_maps, host


def _host_finish(res, host, mus, logvars):
    w1, w2 = host["w1"], host["w2"]
    C1, C2, mu, c0 = host["C1"], host["C2"], host["mu"], host["c0"]

    SG = np.stack([np.asarray(res.results[c]["sgout"], np.float32)
                   for c in range(NCORES)])
    PO = np.stack([np.asarray(res.results[c]["pout"], np.float32)
                   for c in range(NCORES)])
    CA = np.stack([np.asarray(res.results[c]["cacc"], np.float32)
                   for c in range(NCORES)])                 # [NC,128,2]

    H = F // 2

    def rows(arr, lo):  # [NC,128,F-ish] half-slice -> [B,GS]
        return arr[:, :, lo:lo + H].reshape(NCORES, 128, G, GS) \
            .transpose(0, 2, 1, 3).reshape(B, GS)

    Pf = rows(PO, 0).astype(np.float64)
    Pb = rows(PO, H).astype(np.float64)
    Sf = rows(SG, 0).astype(np.float64)
    Sb = rows(SG, H).astype(np.float64)
    Gf = rows(SG[:, :, F:2 * F], 0).astype(np.float64)
    Gb = rows(SG[:, :, F:2 * F], H).astype(np.float64)
    # device CACC: [NC,128,8] = fwd g0..g3, bwd g0..g3 per partition
    cf_rows = CA[:, :, 0:4].transpose(0, 2, 1).reshape(B)
    cb_rows = CA[:, :, 4:8].transpose(0, 2, 1).reshape(B)

    GM = Sf[:, 0:65]
    GI = Gf[:, 0:65]
    bM = Pb[:, 0:65][:, ::-1]
    SIx = (C1 * Sb[:, 0:65][:, ::-1] + C2 * Gb[:, 0:65][:, ::-1])
    tot = (GM * bM * w1 + GI * SIx * w2).sum(axis=1)
    lnP = np.log(np.maximum(tot, 1e-300)) + c0[:, 0] + c0[:, 1] \
        + cf_rows + cb_rows + L * mu
    recon = float(np.mean(-lnP))

    mus = np.asarray(mus, np.float64)
    lv = np.asarray(logvars, np.float64)
    kld = float(np.mean(-0.5 * np.sum(1.0 + lv - mus * mus - np.exp(lv),
                                      axis=1)))
    return np.float32(recon + kld)


def kernel(batch_input, transition_probs, emission_probs, mus, logvars,
           _trace=False, _trace_kwargs=None):
    from concourse.bass_utils import run_bass_kernel_spmd

    if "nc" not in _CACHE:
        _CACHE["nc"] = _build_program()
    nc = _CACHE["nc"]

    in_maps, host = _host_prep(batch_input, transition_probs, emission_probs)
    kw = {}
    if _trace:
        kw["trace"] = True
        kw.update(_trace_kwargs or {})
    res = run_bass_kernel_spmd(nc, in_maps, list(range(NCORES)), **kw)
    _CACHE["last_results"] = res

    return _host_finish(res, host, mus, logvars)


# revision 5
# speedup vs baseline: 1.0786x; 1.0086x over previous
"""Trainium2 Bass kernel for the CNN_PHMM_VAE loss — fused fwd/bwd pHMM, v3.

Like kernel_v2 (fused forward+backward halves, 528-col flat bf16 ops, host
finish), but ALL compute on the Vector engine: GpSimd shares an SBUF port
with DVE and concurrent GpSimd ops were measured to slow DVE ops up to 5x,
which cost more than GpSimd contributed. Also: single TAB stream (the
delete-scan input is rebuilt from S' with a static Q tile), and rescales use
per-partition power-of-2 scales (tensor_scalar 4x mode) — safe because
after host centering the inter-group drift within a partition is << bf16
range.
"""

import sys

import numpy as np

if "/opt/trn_rl_repo" not in sys.path:
    sys.path.insert(0, "/opt/trn_rl_repo")

import ml_dtypes

BF16 = np.dtype(ml_dtypes.bfloat16)

B, K, L, E = 4096, 64, 128, 16
NCORES = 8
BPC = B // NCORES
G = BPC // 128
GS = 66
GB = 2 * GS
F = G * GB                  # 528
N = L // 2
RESCALE_AT = (22, 44)

M2M, M2I, M2D, I2M, I2I, D2M, D2D = range(7)

_CACHE = {}


def _build_program():
    import concourse.bacc as bacc
    import concourse.mybir as mybir
    from concourse.tile import TileContext

    bf16 = mybir.dt.bfloat16
    f32 = mybir.dt.float32
    i32 = mybir.dt.int32
    MULT = mybir.AluOpType.mult
    ADD = mybir.AluOpType.add
    MAX = mybir.AluOpType.max
    SUB = mybir.AluOpType.subtract
    LSR = mybir.AluOpType.logical_shift_right
    AND = mybir.AluOpType.bitwise_and
    X = mybir.AxisListType.X

    nc = bacc.Bacc("TRN2", target_bir_lowering=False, debug=False,
                   num_devices=NCORES)

    tab_h = nc.declare_dram_parameter("tab", [N, 128, F], bf16, isOutput=False)
    c12_h = nc.declare_dram_parameter("c12", [128, 2 * F], bf16, isOutput=False)
    atil_h = nc.declare_dram_parameter("atil", [128, F], bf16, isOutput=False)
    qcol_h = nc.declare_dram_parameter("qcol", [128, F], bf16, isOutput=False)
    big0_h = nc.declare_dram_parameter("big0", [128, 2 * F], bf16, isOutput=False)
    p0_h = nc.declare_dram_parameter("p0", [128, F + 2], bf16, isOutput=False)
    cinit_h = nc.declare_dram_parameter("cinit", [128, 8], f32, isOutput=False)
    pout_h = nc.declare_dram_parameter("pout", [128, F], f32, isOutput=True)
    sgout_h = nc.declare_dram_parameter("sgout", [128, 2 * F], f32, isOutput=True)
    cacc_h = nc.declare_dram_parameter("cacc", [128, 8], f32, isOutput=True)

    with TileContext(nc) as tc:
        with tc.tile_pool(name="consts", bufs=1) as consts, \
             tc.tile_pool(name="state", bufs=1) as state, \
             tc.tile_pool(name="tmps", bufs=2) as tmps, \
             tc.tile_pool(name="stream", bufs=8) as stream:

            # DMA order matters for startup latency: step 1's first three ops
            # need only tab[0], p0, atil, qcol; c12/big0 feed the pre-loop
            # t-pair, which step 1 consumes only at its 4th op.
            tab0 = stream.tile([128, F], bf16, name="tab", tag="tab")
            nc.sync.dma_start(tab0[:], tab_h[0])
            PT = [state.tile([128, F + 2], bf16, name=f"P{j}") for j in range(2)]
            nc.sync.dma_start(PT[0][:], p0_h[:])
            ATIL = consts.tile([128, F], bf16)
            nc.sync.dma_start(ATIL[:], atil_h[:])
            QCOL = consts.tile([128, F], bf16)
            nc.sync.dma_start(QCOL[:], qcol_h[:])
            C12 = consts.tile([128, 2 * F], bf16)
            nc.sync.dma_start(C12[:], c12_h[:])

            # [pad2 | S_e | GIa_e | S_o | GIa_o]
            BIGT = state.tile([128, 2 + 4 * F], bf16)
            nc.vector.memset(BIGT[:, 0:2], 0.0)
            nc.sync.dma_start(BIGT[:, 2:2 + 2 * F], big0_h[:])
            nc.vector.memset(BIGT[:, 2 + 2 * F:2 + 4 * F], 0.0)
            nc.vector.memset(PT[1][:], 0.0)
            CACC = state.tile([128, 8], f32)
            nc.sync.dma_start(CACC[:], cinit_h[:])

            def s_off(h):
                return 2 + 2 * F * h

            def s_half(h):
                o = s_off(h)
                return BIGT[:, o:o + F]

            def gia_half(h):
                o = s_off(h) + F
                return BIGT[:, o:o + F]

            def pair_half(h):
                o = s_off(h)
                return BIGT[:, o:o + 2 * F]

            def s_shift_half(h):
                o = s_off(h)
                return BIGT[:, o - 1:o - 1 + F]

            tpair = tmps.tile([128, 2 * F], bf16, name="tp", tag="tp")
            nc.vector.tensor_tensor(tpair[:], C12[:], pair_half(0), MULT)

            for i in range(1, N + 1):
                prev_h, cur_h = (i + 1) % 2, i % 2
                Pprev, Pcur = PT[prev_h], PT[cur_h]
                if i == 1:
                    tab = tab0
                else:
                    tab = stream.tile([128, F], bf16, name="tab", tag="tab")
                    nc.sync.dma_start(tab[:], tab_h[i - 1])

                # S' = TAB * P[-1]   (tab col0 = 0 -> writes S'[0] = 0)
                nc.vector.tensor_tensor(s_half(cur_h), tab[:],
                                        Pprev[:, 1:F + 1], MULT)
                # qs = Q * S'[-1]
                qs = tmps.tile([128, F], bf16, name="qs", tag="qs")
                nc.vector.tensor_tensor(qs[:], QCOL[:], s_shift_half(cur_h),
                                        MULT)
                # dd = scan(ATIL, qs)
                dd = tmps.tile([128, F], bf16, name="dd", tag="dd")
                nc.vector.tensor_tensor_scan(dd[:], ATIL[:], qs[:],
                                             0.0, MULT, ADD)
                # GIa = t1 + t2 ; h = S' + GIa ; P = h + dd
                nc.vector.tensor_tensor(gia_half(cur_h), tpair[:, 0:F],
                                        tpair[:, F:2 * F], ADD)
                hh = tmps.tile([128, F], bf16, name="hh", tag="hh")
                nc.vector.tensor_tensor(hh[:], s_half(cur_h), gia_half(cur_h),
                                        ADD)
                nc.vector.tensor_tensor(Pcur[:, 2:F + 2], hh[:], dd[:], ADD)

                if i in RESCALE_AT:
                    # power-of-2 rescale per (partition, group, half): one
                    # scale per row-half, broadcast over its 66 columns
                    P3 = Pcur[:, 2:F + 2].rearrange("p (g k) -> p g k", g=8)
                    rm = tmps.tile([128, 8], f32, name="rm", tag="rm")
                    nc.vector.tensor_reduce(rm[:], P3, X, MAX)
                    nc.vector.tensor_scalar_max(rm[:], rm[:], 1e-30)
                    mask = tmps.tile([128, 8], i32, name="msk", tag="msk")
                    nc.vector.tensor_scalar(mask[:], rm.bitcast(i32),
                                            0x7F800000, None, AND)
                    rib = tmps.tile([128, 8], i32, name="rib", tag="rib")
                    nc.vector.tensor_scalar(rib[:], mask[:], -1, 0x7F000000,
                                            MULT, ADD)
                    rinv = tmps.tile([128, 8], f32, name="riv", tag="riv")
                    nc.vector.tensor_copy(rinv.bitcast(i32), rib[:])
                    es = tmps.tile([128, 8], i32, name="es", tag="es")
                    nc.vector.tensor_scalar(es[:], mask[:], 23, None, LSR)
                    ef = tmps.tile([128, 8], f32, name="ef", tag="ef")
                    nc.vector.tensor_copy(ef[:], es[:])
                    el = tmps.tile([128, 8], f32, name="el", tag="el")
                    nc.vector.tensor_scalar(el[:], ef[:], 127.0,
                                            float(np.log(2.0)), SUB, MULT)
                    nc.vector.tensor_tensor(CACC[:], CACC[:], el[:], ADD)
                    rb = tmps.tile([128, 8], bf16, name="rb", tag="rb")
                    nc.vector.tensor_copy(rb[:], rinv[:])
                    sc3 = rb[:, :, None].to_broadcast((128, 8, GS))
                    nc.vector.tensor_tensor(P3, P3, sc3, MULT)
                    o = s_off(cur_h)
                    S3 = BIGT[:, o:o + F].rearrange("p (g k) -> p g k", g=8)
                    G3 = BIGT[:, o + F:o + 2 * F].rearrange(
                        "p (g k) -> p g k", g=8)
                    nc.vector.tensor_tensor(S3, S3, sc3, MULT)
                    nc.vector.tensor_tensor(G3, G3, sc3, MULT)

                # t-pair for step i+1
                tpair = tmps.tile([128, 2 * F], bf16, name="tp", tag="tp")
                nc.vector.tensor_tensor(tpair[:], C12[:], pair_half(cur_h),
                                        MULT)

            fin_h = N % 2
            po = consts.tile([128, F], f32)
            nc.vector.tensor_copy(po[:], PT[fin_h][:, 2:F + 2])
            nc.sync.dma_start(pout_h[:], po[:])
            sg = consts.tile([128, 2 * F], f32)
            nc.vector.tensor_copy(sg[:], pair_half(fin_h))
            nc.sync.dma_start(sgout_h[:], sg[:])
            nc.sync.dma_start(cacc_h[:], CACC[:])

    nc.compile()
    return nc


def _to_pg(arr):
    tail = arr.shape[1:]
    return arr.reshape(NCORES, G, 128, *tail).transpose(
        0, 2, 1, *range(3, 3 + len(tail)))


def _host_prep(batch_input, transition_probs, emission_probs):
    x = np.asarray(batch_input)
    a = np.asarray(transition_probs, np.float64)
    e = np.asarray(emission_probs, np.float64)

    aM2M, aM2I, aM2D = a[:, :, M2M], a[:, :, M2I], a[:, :, M2D]
    aI2M, aI2I = a[:, :, I2M], a[:, :, I2I]
    aD2M, aD2D = a[:, :, D2M], a[:, :, D2D]

    mu = aM2M[:, 1:].mean(axis=1) + e.mean(axis=(1, 2))

    C1 = 0.25 * np.exp(aI2M + aM2I - aM2M - mu[:, None])
    C2 = 0.25 * np.exp(aI2I - mu[:, None])
    Qf = np.zeros((B, 65))
    Qf[:, 1:] = np.exp(aD2M[:, 1:] + aM2D[:, :-1] - aM2M[:, :-1])
    Af = np.zeros((B, 65))
    Af[:, 1:] = np.exp(aD2M[:, 1:] + aD2D[:, :-1] - aD2M[:, :-1])
    Qb = np.zeros((B, 65))
    Qb[:, 0:64] = np.exp(aM2D[:, 0:64] + aD2M[:, 1:65] - aM2M[:, 1:65])
    Ab = np.zeros((B, 65))
    Ab[:, 0:64] = np.exp(aM2D[:, 0:64] + aD2D[:, 1:65] - aM2D[:, 1:65])

    # layout: per row a fwd 66-col block and a bwd 66-col block; on device the
    # four groups' fwd blocks are contiguous (cols 0:264) then the bwd blocks
    def to_col(fwd65, bwd65_by_k):
        f = np.zeros((B, GS))
        f[:, 0:65] = fwd65
        bwd = np.zeros((B, GS))
        bwd[:, 0:65] = bwd65_by_k[:, ::-1]
        return f, bwd

    C1col = to_col(C1, C1)
    C2col = to_col(C2, C2)
    Qcol = to_col(Qf, Qb)
    Acol = to_col(Af, Ab)

    TABf = np.zeros((N, B, GS), np.float32)
    TABb = np.zeros((N, B, GS), np.float32)
    bidx = np.arange(B)[:, None]
    kf = np.arange(64)[None, :]
    kk = (64 - np.arange(1, 65))[None, :]
    for i in range(1, N + 1):
        TABf[i - 1, :, 1:65] = np.exp(
            aM2M[:, 1:65] + e[bidx, kf, x[:, i - 1][:, None]] - mu[:, None])
        TABb[i - 1, :, 1:65] = np.exp(
            aM2M[:, kk[0]] + e[bidx, kk, x[:, L - i][:, None]] - mu[:, None])

    fD0 = np.full((B, 65), -np.inf)
    fD0[:, 1] = aM2D[:, 0]
    fD0[:, 2:] = aM2D[:, 0:1] + np.cumsum(aD2D[:, 1:64], axis=1)
    gm0 = np.full((B, 65), -np.inf)
    gm0[:, 0] = aM2M[:, 0]
    gd0 = fD0 + aD2M
    gd0[:, 0] = -np.inf
    c0f = np.maximum(gm0.max(axis=1), gd0.max(axis=1))
    GM0 = np.exp(gm0 - c0f[:, None])
    GD0 = np.exp(gd0 - c0f[:, None])
    lnbD = np.empty((B, 65))
    lnbD[:, 64] = aD2M[:, 64]
    rev_cs = np.cumsum(aD2D[:, ::-1][:, 1:65], axis=1)[:, ::-1]
    lnbD[:, 0:64] = rev_cs + aD2M[:, 64:65]
    lnbM = np.empty((B, 65))
    lnbM[:, 64] = aM2M[:, 64]
    lnbM[:, 0:64] = aM2D[:, 0:64] + lnbD[:, 1:65]
    c0b = lnbM.max(axis=1)
    BP0 = np.exp(lnbM - c0b[:, None])
    SIa0 = np.zeros((B, 65))
    SIa0[:, 64] = np.exp(aM2I[:, 64] + aI2M[:, 64] - aI2I[:, 64] - c0b)

    P0 = (np.pad(GM0 + GD0, ((0, 0), (0, 1))),
          np.pad(BP0[:, ::-1], ((0, 0), (0, 1))))
    S0 = (np.pad(GM0, ((0, 0), (0, 1))), np.zeros((B, GS)))
    GIa0 = (np.zeros((B, GS)), np.pad(SIa0[:, ::-1], ((0, 0), (0, 1))))

    w1 = np.exp(-aM2M)
    w2 = np.exp(mu[:, None] - (aI2M + aM2I)) / 0.25

    def pack(fb):
        f, bwd = fb
        return np.concatenate(
            [_to_pg(f).reshape(NCORES, 128, F // 2),
             _to_pg(bwd).reshape(NCORES, 128, F // 2)], axis=2)

    c12 = np.concatenate([pack(C1col), pack(C2col)], axis=2)
    atil = pack(Acol)
    qcol = pack(Qcol)
    big0 = np.concatenate([pack(S0), pack(GIa0)], axis=2)
    p0 = pack((P0[0], P0[1]))
    p0 = np.concatenate([np.zeros((NCORES, 128, 2)), p0], axis=2)
    # per-partition CACC init: groups of one partition share the scale terms
    # only through the host finish, which reads per-row c0 separately; device
    # CACC tracks only the (shared) rescale exponents, init 0.
    cinit = np.zeros((NCORES, 128, 8), np.float32)
    tab = np.concatenate([
        TABf.reshape(N, NCORES, G, 128, GS).transpose(1, 0, 3, 2, 4)
            .reshape(NCORES, N, 128, F // 2),
        TABb.reshape(N, NCORES, G, 128, GS).transpose(1, 0, 3, 2, 4)
            .reshape(NCORES, N, 128, F // 2)], axis=3)

    in_maps = []
    for c in range(NCORES):
        in_maps.append({
            "tab": np.ascontiguousarray(tab[c]).astype(BF16),
            "c12": np.ascontiguousarray(c12[c]).astype(BF16),
            "atil": np.ascontiguousarray(atil[c]).astype(BF16),
            "qcol": np.ascontiguousarray(qcol[c]).astype(BF16),
            "big0": np.ascontiguousarray(big0[c]).astype(BF16),
            "p0": np.ascontiguousarray(p0[c]).astype(BF16),
            "cinit": np.ascontiguousarray(cinit[c]),
        })
    host = dict(w1=w1, w2=w2, C1=C1, C2=C2, mu=mu,
                c0=np.stack([c0f, c0b], axis=1))
    return in_maps, host


def _host_finish(res, host, mus, logvars):
    w1, w2 = host["w1"], host["w2"]
    C1, C2, mu, c0 = host["C1"], host["C2"], host["mu"], host["c0"]

    SG = np.stack([np.asarray(res.results[c]["sgout"], np.float32)
                   for c in range(NCORES)])
    PO = np.stack([np.asarray(res.results[c]["pout"], np.float32)
                   for c in range(NCORES)])
    CA = np.stack([np.asarray(res.results[c]["cacc"], np.float32)
                   for c in range(NCORES)])                 # [NC,128,2]

    H = F // 2

    def rows(arr, lo):  # [NC,128,F-ish] half-slice -> [B,GS]
        return arr[:, :, lo:lo + H].reshape(NCORES, 128, G, GS) \
            .transpose(0, 2, 1, 3).reshape(B, GS)

    Pf = rows(PO, 0).astype(np.float64)
    Pb = rows(PO, H).astype(np.float64)
    Sf = rows(SG, 0).astype(np.float64)
    Sb = rows(SG, H).astype(np.float64)
    Gf = rows(SG[:, :, F:2 * F], 0).astype(np.float64)
    Gb = rows(SG[:, :, F:2 * F], H).astype(np.float64)
    # device CACC: [NC,128,8] = fwd g0..g3, bwd g0..g3 per partition
    cf_rows = CA[:, :, 0:4].transpose(0, 2, 1).reshape(B)
    cb_rows = CA[:, :, 4:8].transpose(0, 2, 1).reshape(B)

    GM = Sf[:, 0:65]
    GI = Gf[:, 0:65]
    bM = Pb[:, 0:65][:, ::-1]
    SIx = (C1 * Sb[:, 0:65][:, ::-1] + C2 * Gb[:, 0:65][:, ::-1])
    tot = (GM * bM * w1 + GI * SIx * w2).sum(axis=1)
    lnP = np.log(np.maximum(tot, 1e-300)) + c0[:, 0] + c0[:, 1] \
        + cf_rows + cb_rows + L * mu
    recon = float(np.mean(-lnP))

    mus = np.asarray(mus, np.float64)
    lv = np.asarray(logvars, np.float64)
    kld = float(np.mean(-0.5 * np.sum(1.0 + lv - mus * mus - np.exp(lv),
                                      axis=1)))
    return np.float32(recon + kld)


def kernel(batch_input, transition_probs, emission_probs, mus, logvars,
           _trace=False, _trace_kwargs=None):
    from concourse.bass_utils import run_bass_kernel_spmd

    if "nc" not in _CACHE:
        _CACHE["nc"] = _build_program()
    nc = _CACHE["nc"]

    in_maps, host = _host_prep(batch_input, transition_probs, emission_probs)
    kw = {}
    if _trace:
        kw["trace"] = True
        kw.update(_trace_kwargs or {})
    res = run_bass_kernel_spmd(nc, in_maps, list(range(NCORES)), **kw)
    _CACHE["last_results"] = res

    return _host_finish(res, host, mus, logvars)


# revision 6
# speedup vs baseline: 1.0912x; 1.0117x over previous
"""Trainium2 Bass kernel for the CNN_PHMM_VAE loss — fused fwd/bwd pHMM, v3.

Like kernel_v2 (fused forward+backward halves, 528-col flat bf16 ops, host
finish), but ALL compute on the Vector engine: GpSimd shares an SBUF port
with DVE and concurrent GpSimd ops were measured to slow DVE ops up to 5x,
which cost more than GpSimd contributed. Also: single TAB stream (the
delete-scan input is rebuilt from S' with a static Q tile), and rescales use
per-partition power-of-2 scales (tensor_scalar 4x mode) — safe because
after host centering the inter-group drift within a partition is << bf16
range.
"""

import sys

import numpy as np

if "/opt/trn_rl_repo" not in sys.path:
    sys.path.insert(0, "/opt/trn_rl_repo")

import ml_dtypes

BF16 = np.dtype(ml_dtypes.bfloat16)

B, K, L, E = 4096, 64, 128, 16
NCORES = 8
BPC = B // NCORES
G = BPC // 128
GS = 66
GB = 2 * GS
F = G * GB                  # 528
N = L // 2
RESCALE_AT = (22, 44)

M2M, M2I, M2D, I2M, I2I, D2M, D2D = range(7)

_CACHE = {}


def _build_program():
    import concourse.bacc as bacc
    import concourse.mybir as mybir
    from concourse.tile import TileContext

    bf16 = mybir.dt.bfloat16
    f32 = mybir.dt.float32
    i32 = mybir.dt.int32
    MULT = mybir.AluOpType.mult
    ADD = mybir.AluOpType.add
    MAX = mybir.AluOpType.max
    SUB = mybir.AluOpType.subtract
    LSR = mybir.AluOpType.logical_shift_right
    AND = mybir.AluOpType.bitwise_and
    X = mybir.AxisListType.X

    nc = bacc.Bacc("TRN2", target_bir_lowering=False, debug=False,
                   num_devices=NCORES)

    tab_h = nc.declare_dram_parameter("tab", [N, 128, F], bf16, isOutput=False)
    c12_h = nc.declare_dram_parameter("c12", [128, 2 * F], bf16, isOutput=False)
    aq_h = nc.declare_dram_parameter("aq", [128, 2 * F], bf16, isOutput=False)
    big0_h = nc.declare_dram_parameter("big0", [128, 2 * F], bf16, isOutput=False)
    p0_h = nc.declare_dram_parameter("p0", [128, F + 2], bf16, isOutput=False)
    cinit_h = nc.declare_dram_parameter("cinit", [128, 8], f32, isOutput=False)
    pout_h = nc.declare_dram_parameter("pout", [128, F], bf16, isOutput=True)
    sgout_h = nc.declare_dram_parameter("sgout", [128, 2 * F], bf16, isOutput=True)
    cacc_h = nc.declare_dram_parameter("cacc", [128, 8], f32, isOutput=True)

    with TileContext(nc) as tc:
        with tc.tile_pool(name="consts", bufs=1) as consts, \
             tc.tile_pool(name="state", bufs=1) as state, \
             tc.tile_pool(name="tmps", bufs=2) as tmps, \
             tc.tile_pool(name="stream", bufs=8) as stream:

            # DMA order matters for startup latency: step 1's first three ops
            # need only tab[0], p0, atil, qcol; c12/big0 feed the pre-loop
            # t-pair, which step 1 consumes only at its 4th op.
            tab0 = stream.tile([128, F], bf16, name="tab", tag="tab")
            nc.sync.dma_start(tab0[:], tab_h[0])
            PT = [state.tile([128, F + 2], bf16, name=f"P{j}") for j in range(2)]
            nc.sync.dma_start(PT[0][:], p0_h[:])
            AQ = consts.tile([128, 2 * F], bf16)
            nc.sync.dma_start(AQ[:], aq_h[:])
            ATIL = AQ[:, 0:F]
            QCOL = AQ[:, F:2 * F]
            C12 = consts.tile([128, 2 * F], bf16)
            nc.sync.dma_start(C12[:], c12_h[:])

            # [pad2 | S_e | GIa_e | S_o | GIa_o]
            BIGT = state.tile([128, 2 + 4 * F], bf16)
            nc.vector.memset(BIGT[:, 0:2], 0.0)
            nc.sync.dma_start(BIGT[:, 2:2 + 2 * F], big0_h[:])
            nc.vector.memset(BIGT[:, 2 + 2 * F:2 + 4 * F], 0.0)
            nc.vector.memset(PT[1][:], 0.0)
            CACC = state.tile([128, 8], f32)
            nc.sync.dma_start(CACC[:], cinit_h[:])

            def s_off(h):
                return 2 + 2 * F * h

            def s_half(h):
                o = s_off(h)
                return BIGT[:, o:o + F]

            def gia_half(h):
                o = s_off(h) + F
                return BIGT[:, o:o + F]

            def pair_half(h):
                o = s_off(h)
                return BIGT[:, o:o + 2 * F]

            def s_shift_half(h):
                o = s_off(h)
                return BIGT[:, o - 1:o - 1 + F]

            tpair = tmps.tile([128, 2 * F], bf16, name="tp", tag="tp")
            nc.vector.tensor_tensor(tpair[:], C12[:], pair_half(0), MULT)

            for i in range(1, N + 1):
                prev_h, cur_h = (i + 1) % 2, i % 2
                Pprev, Pcur = PT[prev_h], PT[cur_h]
                if i == 1:
                    tab = tab0
                else:
                    tab = stream.tile([128, F], bf16, name="tab", tag="tab")
                    nc.sync.dma_start(tab[:], tab_h[i - 1])

                # S' = TAB * P[-1]   (tab col0 = 0 -> writes S'[0] = 0)
                nc.vector.tensor_tensor(s_half(cur_h), tab[:],
                                        Pprev[:, 1:F + 1], MULT)
                # qs = Q * S'[-1]
                qs = tmps.tile([128, F], bf16, name="qs", tag="qs")
                nc.vector.tensor_tensor(qs[:], QCOL, s_shift_half(cur_h),
                                        MULT)
                # dd = scan(ATIL, qs)
                dd = tmps.tile([128, F], bf16, name="dd", tag="dd")
                nc.vector.tensor_tensor_scan(dd[:], ATIL, qs[:],
                                             0.0, MULT, ADD)
                # GIa = t1 + t2 ; h = S' + GIa ; P = h + dd
                nc.vector.tensor_tensor(gia_half(cur_h), tpair[:, 0:F],
                                        tpair[:, F:2 * F], ADD)
                hh = tmps.tile([128, F], bf16, name="hh", tag="hh")
                nc.vector.tensor_tensor(hh[:], s_half(cur_h), gia_half(cur_h),
                                        ADD)
                nc.vector.tensor_tensor(Pcur[:, 2:F + 2], hh[:], dd[:], ADD)

                if i in RESCALE_AT:
                    # power-of-2 rescale per (partition, group, half): one
                    # scale per row-half, broadcast over its 66 columns
                    P3 = Pcur[:, 2:F + 2].rearrange("p (g k) -> p g k", g=8)
                    rm = tmps.tile([128, 8], f32, name="rm", tag="rm")
                    nc.vector.tensor_reduce(rm[:], P3, X, MAX)
                    nc.vector.tensor_scalar_max(rm[:], rm[:], 1e-30)
                    mask = tmps.tile([128, 8], i32, name="msk", tag="msk")
                    nc.vector.tensor_scalar(mask[:], rm.bitcast(i32),
                                            0x7F800000, None, AND)
                    rib = tmps.tile([128, 8], i32, name="rib", tag="rib")
                    nc.vector.tensor_scalar(rib[:], mask[:], -1, 0x7F000000,
                                            MULT, ADD)
                    rinv = tmps.tile([128, 8], f32, name="riv", tag="riv")
                    nc.vector.tensor_copy(rinv.bitcast(i32), rib[:])
                    es = tmps.tile([128, 8], i32, name="es", tag="es")
                    nc.vector.tensor_scalar(es[:], mask[:], 23, None, LSR)
                    ef = tmps.tile([128, 8], f32, name="ef", tag="ef")
                    nc.vector.tensor_copy(ef[:], es[:])
                    el = tmps.tile([128, 8], f32, name="el", tag="el")
                    nc.vector.tensor_scalar(el[:], ef[:], 127.0,
                                            float(np.log(2.0)), SUB, MULT)
                    nc.vector.tensor_tensor(CACC[:], CACC[:], el[:], ADD)
                    rb = tmps.tile([128, 8], bf16, name="rb", tag="rb")
                    nc.vector.tensor_copy(rb[:], rinv[:])
                    sc3 = rb[:, :, None].to_broadcast((128, 8, GS))
                    nc.vector.tensor_tensor(P3, P3, sc3, MULT)
                    o = s_off(cur_h)
                    S3 = BIGT[:, o:o + F].rearrange("p (g k) -> p g k", g=8)
                    G3 = BIGT[:, o + F:o + 2 * F].rearrange(
                        "p (g k) -> p g k", g=8)
                    nc.vector.tensor_tensor(S3, S3, sc3, MULT)
                    nc.vector.tensor_tensor(G3, G3, sc3, MULT)

                # t-pair for step i+1
                tpair = tmps.tile([128, 2 * F], bf16, name="tp", tag="tp")
                nc.vector.tensor_tensor(tpair[:], C12[:], pair_half(cur_h),
                                        MULT)

            fin_h = N % 2
            nc.sync.dma_start(pout_h[:], PT[fin_h][:, 2:F + 2])
            nc.sync.dma_start(sgout_h[:, 0:F], s_half(fin_h))
            nc.sync.dma_start(sgout_h[:, F:2 * F], gia_half(fin_h))
            nc.sync.dma_start(cacc_h[:], CACC[:])

    nc.compile()
    return nc


def _to_pg(arr):
    tail = arr.shape[1:]
    return arr.reshape(NCORES, G, 128, *tail).transpose(
        0, 2, 1, *range(3, 3 + len(tail)))


def _host_prep(batch_input, transition_probs, emission_probs):
    x = np.asarray(batch_input)
    a = np.asarray(transition_probs, np.float64)
    e = np.asarray(emission_probs, np.float64)

    aM2M, aM2I, aM2D = a[:, :, M2M], a[:, :, M2I], a[:, :, M2D]
    aI2M, aI2I = a[:, :, I2M], a[:, :, I2I]
    aD2M, aD2D = a[:, :, D2M], a[:, :, D2D]

    mu = aM2M[:, 1:].mean(axis=1) + e.mean(axis=(1, 2))

    C1 = 0.25 * np.exp(aI2M + aM2I - aM2M - mu[:, None])
    C2 = 0.25 * np.exp(aI2I - mu[:, None])
    Qf = np.zeros((B, 65))
    Qf[:, 1:] = np.exp(aD2M[:, 1:] + aM2D[:, :-1] - aM2M[:, :-1])
    Af = np.zeros((B, 65))
    Af[:, 1:] = np.exp(aD2M[:, 1:] + aD2D[:, :-1] - aD2M[:, :-1])
    Qb = np.zeros((B, 65))
    Qb[:, 0:64] = np.exp(aM2D[:, 0:64] + aD2M[:, 1:65] - aM2M[:, 1:65])
    Ab = np.zeros((B, 65))
    Ab[:, 0:64] = np.exp(aM2D[:, 0:64] + aD2D[:, 1:65] - aM2D[:, 1:65])

    # layout: per row a fwd 66-col block and a bwd 66-col block; on device the
    # four groups' fwd blocks are contiguous (cols 0:264) then the bwd blocks
    def to_col(fwd65, bwd65_by_k):
        f = np.zeros((B, GS))
        f[:, 0:65] = fwd65
        bwd = np.zeros((B, GS))
        bwd[:, 0:65] = bwd65_by_k[:, ::-1]
        return f, bwd

    C1col = to_col(C1, C1)
    C2col = to_col(C2, C2)
    Qcol = to_col(Qf, Qb)
    Acol = to_col(Af, Ab)

    TABf = np.zeros((N, B, GS), np.float32)
    TABb = np.zeros((N, B, GS), np.float32)
    bidx = np.arange(B)[:, None]
    kf = np.arange(64)[None, :]
    kk = (64 - np.arange(1, 65))[None, :]
    for i in range(1, N + 1):
        TABf[i - 1, :, 1:65] = np.exp(
            aM2M[:, 1:65] + e[bidx, kf, x[:, i - 1][:, None]] - mu[:, None])
        TABb[i - 1, :, 1:65] = np.exp(
            aM2M[:, kk[0]] + e[bidx, kk, x[:, L - i][:, None]] - mu[:, None])

    fD0 = np.full((B, 65), -np.inf)
    fD0[:, 1] = aM2D[:, 0]
    fD0[:, 2:] = aM2D[:, 0:1] + np.cumsum(aD2D[:, 1:64], axis=1)
    gm0 = np.full((B, 65), -np.inf)
    gm0[:, 0] = aM2M[:, 0]
    gd0 = fD0 + aD2M
    gd0[:, 0] = -np.inf
    c0f = np.maximum(gm0.max(axis=1), gd0.max(axis=1))
    GM0 = np.exp(gm0 - c0f[:, None])
    GD0 = np.exp(gd0 - c0f[:, None])
    lnbD = np.empty((B, 65))
    lnbD[:, 64] = aD2M[:, 64]
    rev_cs = np.cumsum(aD2D[:, ::-1][:, 1:65], axis=1)[:, ::-1]
    lnbD[:, 0:64] = rev_cs + aD2M[:, 64:65]
    lnbM = np.empty((B, 65))
    lnbM[:, 64] = aM2M[:, 64]
    lnbM[:, 0:64] = aM2D[:, 0:64] + lnbD[:, 1:65]
    c0b = lnbM.max(axis=1)
    BP0 = np.exp(lnbM - c0b[:, None])
    SIa0 = np.zeros((B, 65))
    SIa0[:, 64] = np.exp(aM2I[:, 64] + aI2M[:, 64] - aI2I[:, 64] - c0b)

    P0 = (np.pad(GM0 + GD0, ((0, 0), (0, 1))),
          np.pad(BP0[:, ::-1], ((0, 0), (0, 1))))
    S0 = (np.pad(GM0, ((0, 0), (0, 1))), np.zeros((B, GS)))
    GIa0 = (np.zeros((B, GS)), np.pad(SIa0[:, ::-1], ((0, 0), (0, 1))))

    w1 = np.exp(-aM2M)
    w2 = np.exp(mu[:, None] - (aI2M + aM2I)) / 0.25

    def pack(fb):
        f, bwd = fb
        return np.concatenate(
            [_to_pg(f).reshape(NCORES, 128, F // 2),
             _to_pg(bwd).reshape(NCORES, 128, F // 2)], axis=2)

    c12 = np.concatenate([pack(C1col), pack(C2col)], axis=2)
    atil = pack(Acol)
    qcol = pack(Qcol)
    big0 = np.concatenate([pack(S0), pack(GIa0)], axis=2)
    p0 = pack((P0[0], P0[1]))
    p0 = np.concatenate([np.zeros((NCORES, 128, 2)), p0], axis=2)
    # per-partition CACC init: groups of one partition share the scale terms
    # only through the host finish, which reads per-row c0 separately; device
    # CACC tracks only the (shared) rescale exponents, init 0.
    cinit = np.zeros((NCORES, 128, 8), np.float32)
    tab = np.concatenate([
        TABf.reshape(N, NCORES, G, 128, GS).transpose(1, 0, 3, 2, 4)
            .reshape(NCORES, N, 128, F // 2),
        TABb.reshape(N, NCORES, G, 128, GS).transpose(1, 0, 3, 2, 4)
            .reshape(NCORES, N, 128, F // 2)], axis=3)

    in_maps = []
    for c in range(NCORES):
        in_maps.append({
            "tab": np.ascontiguousarray(tab[c]).astype(BF16),
            "c12": np.ascontiguousarray(c12[c]).astype(BF16),
            "aq": np.ascontiguousarray(
                np.concatenate([atil[c], qcol[c]], axis=1)).astype(BF16),
            "big0": np.ascontiguousarray(big0[c]).astype(BF16),
            "p0": np.ascontiguousarray(p0[c]).astype(BF16),
            "cinit": np.ascontiguousarray(cinit[c]),
        })
    host = dict(w1=w1, w2=w2, C1=C1, C2=C2, mu=mu,
                c0=np.stack([c0f, c0b], axis=1))
    return in_maps, host


def _host_finish(res, host, mus, logvars):
    w1, w2 = host["w1"], host["w2"]
    C1, C2, mu, c0 = host["C1"], host["C2"], host["mu"], host["c0"]

    SG = np.stack([np.asarray(res.results[c]["sgout"]).astype(np.float32)
                   for c in range(NCORES)])
    PO = np.stack([np.asarray(res.results[c]["pout"]).astype(np.float32)
                   for c in range(NCORES)])
    CA = np.stack([np.asarray(res.results[c]["cacc"], np.float32)
                   for c in range(NCORES)])                 # [NC,128,2]

    H = F // 2

    def rows(arr, lo):  # [NC,128,F-ish] half-slice -> [B,GS]
        return arr[:, :, lo:lo + H].reshape(NCORES, 128, G, GS) \
            .transpose(0, 2, 1, 3).reshape(B, GS)

    Pf = rows(PO, 0).astype(np.float64)
    Pb = rows(PO, H).astype(np.float64)
    Sf = rows(SG, 0).astype(np.float64)
    Sb = rows(SG, H).astype(np.float64)
    Gf = rows(SG[:, :, F:2 * F], 0).astype(np.float64)
    Gb = rows(SG[:, :, F:2 * F], H).astype(np.float64)
    # device CACC: [NC,128,8] = fwd g0..g3, bwd g0..g3 per partition
    cf_rows = CA[:, :, 0:4].transpose(0, 2, 1).reshape(B)
    cb_rows = CA[:, :, 4:8].transpose(0, 2, 1).reshape(B)

    GM = Sf[:, 0:65]
    GI = Gf[:, 0:65]
    bM = Pb[:, 0:65][:, ::-1]
    SIx = (C1 * Sb[:, 0:65][:, ::-1] + C2 * Gb[:, 0:65][:, ::-1])
    tot = (GM * bM * w1 + GI * SIx * w2).sum(axis=1)
    lnP = np.log(np.maximum(tot, 1e-300)) + c0[:, 0] + c0[:, 1] \
        + cf_rows + cb_rows + L * mu
    recon = float(np.mean(-lnP))

    mus = np.asarray(mus, np.float64)
    lv = np.asarray(logvars, np.float64)
    kld = float(np.mean(-0.5 * np.sum(1.0 + lv - mus * mus - np.exp(lv),
                                      axis=1)))
    return np.float32(recon + kld)


def kernel(batch_input, transition_probs, emission_probs, mus, logvars,
           _trace=False, _trace_kwargs=None):
    from concourse.bass_utils import run_bass_kernel_spmd

    if "nc" not in _CACHE:
        _CACHE["nc"] = _build_program()
    nc = _CACHE["nc"]

    in_maps, host = _host_prep(batch_input, transition_probs, emission_probs)
    kw = {}
    if _trace:
        kw["trace"] = True
        kw.update(_trace_kwargs or {})
    res = run_bass_kernel_spmd(nc, in_maps, list(range(NCORES)), **kw)
    _CACHE["last_results"] = res

    return _host_finish(res, host, mus, logvars)


# revision 8
# speedup vs baseline: 1.0918x; 1.0006x over previous
"""Trainium2 Bass kernel for the CNN_PHMM_VAE loss — fused fwd/bwd pHMM, v3.

Like kernel_v2 (fused forward+backward halves, 528-col flat bf16 ops, host
finish), but ALL compute on the Vector engine: GpSimd shares an SBUF port
with DVE and concurrent GpSimd ops were measured to slow DVE ops up to 5x,
which cost more than GpSimd contributed. Also: single TAB stream (the
delete-scan input is rebuilt from S' with a static Q tile), and rescales use
per-partition power-of-2 scales (tensor_scalar 4x mode) — safe because
after host centering the inter-group drift within a partition is << bf16
range.
"""

import sys

import numpy as np

if "/opt/trn_rl_repo" not in sys.path:
    sys.path.insert(0, "/opt/trn_rl_repo")

import ml_dtypes

BF16 = np.dtype(ml_dtypes.bfloat16)

B, K, L, E = 4096, 64, 128, 16
NCORES = 8
BPC = B // NCORES
G = BPC // 128
GS = 66
GB = 2 * GS
F = G * GB                  # 528
N = L // 2
RESCALE_AT = (22, 44)

M2M, M2I, M2D, I2M, I2I, D2M, D2D = range(7)

_CACHE = {}


def _build_program():
    import concourse.bacc as bacc
    import concourse.mybir as mybir
    from concourse.tile import TileContext

    bf16 = mybir.dt.bfloat16
    f32 = mybir.dt.float32
    i32 = mybir.dt.int32
    MULT = mybir.AluOpType.mult
    ADD = mybir.AluOpType.add
    MAX = mybir.AluOpType.max
    SUB = mybir.AluOpType.subtract
    LSR = mybir.AluOpType.logical_shift_right
    AND = mybir.AluOpType.bitwise_and
    X = mybir.AxisListType.X

    nc = bacc.Bacc("TRN2", target_bir_lowering=False, debug=False,
                   num_devices=NCORES)

    tab_h = nc.declare_dram_parameter("tab", [N, 128, F], bf16, isOutput=False)
    c12_h = nc.declare_dram_parameter("c12", [128, 2 * F], bf16, isOutput=False)
    aq_h = nc.declare_dram_parameter("aq", [128, 2 * F], bf16, isOutput=False)
    big0_h = nc.declare_dram_parameter("big0", [128, 2 * F], bf16, isOutput=False)
    p0_h = nc.declare_dram_parameter("p0", [128, F + 2], bf16, isOutput=False)
    cinit_h = nc.declare_dram_parameter("cinit", [128, 8], f32, isOutput=False)
    pout_h = nc.declare_dram_parameter("pout", [128, F], bf16, isOutput=True)
    sgout_h = nc.declare_dram_parameter("sgout", [128, 2 * F], bf16, isOutput=True)
    cacc_h = nc.declare_dram_parameter("cacc", [128, 8], f32, isOutput=True)

    with TileContext(nc) as tc:
        with tc.tile_pool(name="consts", bufs=1) as consts, \
             tc.tile_pool(name="state", bufs=1) as state, \
             tc.tile_pool(name="tmps", bufs=2) as tmps, \
             tc.tile_pool(name="stream", bufs=8) as stream:

            # DMA order matters for startup latency: step 1's first three ops
            # need only tab[0], p0, atil, qcol; c12/big0 feed the pre-loop
            # t-pair, which step 1 consumes only at its 4th op.
            tab0 = stream.tile([128, F], bf16, name="tab", tag="tab")
            nc.sync.dma_start(tab0[:], tab_h[0])
            PT = [state.tile([128, F + 2], bf16, name=f"P{j}") for j in range(2)]
            nc.sync.dma_start(PT[0][:], p0_h[:])
            AQ = consts.tile([128, 2 * F], bf16)
            nc.sync.dma_start(AQ[:], aq_h[:])
            ATIL = AQ[:, 0:F]
            QCOL = AQ[:, F:2 * F]
            C12 = consts.tile([128, 2 * F], bf16)
            nc.sync.dma_start(C12[:], c12_h[:])

            # [pad2 | S_e | GIa_e | S_o | GIa_o]
            BIGT = state.tile([128, 2 + 4 * F], bf16)
            nc.vector.memset(BIGT[:, 0:2], 0.0)
            nc.sync.dma_start(BIGT[:, 2:2 + 2 * F], big0_h[:])
            nc.vector.memset(BIGT[:, 2 + 2 * F:2 + 4 * F], 0.0)
            nc.vector.memset(PT[1][:], 0.0)
            CACC = state.tile([128, 8], f32)
            nc.sync.dma_start(CACC[:], cinit_h[:])

            def s_off(h):
                return 2 + 2 * F * h

            def s_half(h):
                o = s_off(h)
                return BIGT[:, o:o + F]

            def gia_half(h):
                o = s_off(h) + F
                return BIGT[:, o:o + F]

            def pair_half(h):
                o = s_off(h)
                return BIGT[:, o:o + 2 * F]

            def s_shift_half(h):
                o = s_off(h)
                return BIGT[:, o - 1:o - 1 + F]

            tpair = tmps.tile([128, 2 * F], bf16, name="tp", tag="tp")
            nc.vector.tensor_tensor(tpair[:], C12[:], pair_half(0), MULT)

            for i in range(1, N + 1):
                prev_h, cur_h = (i + 1) % 2, i % 2
                Pprev, Pcur = PT[prev_h], PT[cur_h]
                if i == 1:
                    tab = tab0
                else:
                    tab = stream.tile([128, F], bf16, name="tab", tag="tab")
                    nc.sync.dma_start(tab[:], tab_h[i - 1])

                # S' = TAB * P[-1]   (tab col0 = 0 -> writes S'[0] = 0)
                nc.vector.tensor_tensor(s_half(cur_h), tab[:],
                                        Pprev[:, 1:F + 1], MULT)
                # qs = Q * S'[-1]
                qs = tmps.tile([128, F], bf16, name="qs", tag="qs")
                nc.vector.tensor_tensor(qs[:], QCOL, s_shift_half(cur_h),
                                        MULT)
                # dd = scan(ATIL, qs)
                dd = tmps.tile([128, F], bf16, name="dd", tag="dd")
                nc.vector.tensor_tensor_scan(dd[:], ATIL, qs[:],
                                             0.0, MULT, ADD)
                # GIa = t1 + t2 ; h = S' + GIa ; P = h + dd
                nc.vector.tensor_tensor(gia_half(cur_h), tpair[:, 0:F],
                                        tpair[:, F:2 * F], ADD)
                hh = tmps.tile([128, F], bf16, name="hh", tag="hh")
                nc.vector.tensor_tensor(hh[:], s_half(cur_h), gia_half(cur_h),
                                        ADD)
                nc.vector.tensor_tensor(Pcur[:, 2:F + 2], hh[:], dd[:], ADD)

                if i in RESCALE_AT:
                    # power-of-2 rescale per (partition, group, half): one
                    # scale per row-half, broadcast over its 66 columns
                    P3 = Pcur[:, 2:F + 2].rearrange("p (g k) -> p g k", g=8)
                    rm = tmps.tile([128, 8], f32, name="rm", tag="rm")
                    nc.vector.tensor_reduce(rm[:], P3, X, MAX)
                    nc.vector.tensor_scalar_max(rm[:], rm[:], 1e-30)
                    mask = tmps.tile([128, 8], i32, name="msk", tag="msk")
                    nc.vector.tensor_scalar(mask[:], rm.bitcast(i32),
                                            0x7F800000, None, AND)
                    rib = tmps.tile([128, 8], i32, name="rib", tag="rib")
                    nc.vector.tensor_scalar(rib[:], mask[:], -1, 0x7F000000,
                                            MULT, ADD)
                    rinv = tmps.tile([128, 8], f32, name="riv", tag="riv")
                    nc.vector.tensor_copy(rinv.bitcast(i32), rib[:])
                    es = tmps.tile([128, 8], i32, name="es", tag="es")
                    nc.vector.tensor_scalar(es[:], mask[:], 23, None, LSR)
                    ef = tmps.tile([128, 8], f32, name="ef", tag="ef")
                    nc.vector.tensor_copy(ef[:], es[:])
                    el = tmps.tile([128, 8], f32, name="el", tag="el")
                    nc.vector.tensor_scalar(el[:], ef[:], 127.0,
                                            float(np.log(2.0)), SUB, MULT)
                    nc.vector.tensor_tensor(CACC[:], CACC[:], el[:], ADD)
                    rb = tmps.tile([128, 8], bf16, name="rb", tag="rb")
                    nc.vector.tensor_copy(rb[:], rinv[:])
                    sc3 = rb[:, :, None].to_broadcast((128, 8, GS))
                    nc.vector.tensor_tensor(P3, P3, sc3, MULT)
                    o = s_off(cur_h)
                    S3 = BIGT[:, o:o + F].rearrange("p (g k) -> p g k", g=8)
                    G3 = BIGT[:, o + F:o + 2 * F].rearrange(
                        "p (g k) -> p g k", g=8)
                    nc.vector.tensor_tensor(S3, S3, sc3, MULT)
                    nc.vector.tensor_tensor(G3, G3, sc3, MULT)

                # t-pair for step i+1
                tpair = tmps.tile([128, 2 * F], bf16, name="tp", tag="tp")
                nc.vector.tensor_tensor(tpair[:], C12[:], pair_half(cur_h),
                                        MULT)

            fin_h = N % 2
            nc.sync.dma_start(pout_h[:], PT[fin_h][:, 2:F + 2])
            nc.sync.dma_start(sgout_h[:, 0:F], s_half(fin_h))
            nc.sync.dma_start(sgout_h[:, F:2 * F], gia_half(fin_h))
            nc.sync.dma_start(cacc_h[:], CACC[:])

    nc.compile()
    return nc


def _to_pg(arr):
    tail = arr.shape[1:]
    return arr.reshape(NCORES, G, 128, *tail).transpose(
        0, 2, 1, *range(3, 3 + len(tail)))


def _host_prep(batch_input, transition_probs, emission_probs):
    x = np.asarray(batch_input)
    a = np.asarray(transition_probs, np.float64)
    e = np.asarray(emission_probs, np.float64)

    aM2M, aM2I, aM2D = a[:, :, M2M], a[:, :, M2I], a[:, :, M2D]
    aI2M, aI2I = a[:, :, I2M], a[:, :, I2I]
    aD2M, aD2D = a[:, :, D2M], a[:, :, D2D]

    mu = aM2M[:, 1:].mean(axis=1) + e.mean(axis=(1, 2))

    C1 = 0.25 * np.exp(aI2M + aM2I - aM2M - mu[:, None])
    C2 = 0.25 * np.exp(aI2I - mu[:, None])
    Qf = np.zeros((B, 65))
    Qf[:, 1:] = np.exp(aD2M[:, 1:] + aM2D[:, :-1] - aM2M[:, :-1])
    Af = np.zeros((B, 65))
    Af[:, 1:] = np.exp(aD2M[:, 1:] + aD2D[:, :-1] - aD2M[:, :-1])
    Qb = np.zeros((B, 65))
    Qb[:, 0:64] = np.exp(aM2D[:, 0:64] + aD2M[:, 1:65] - aM2M[:, 1:65])
    Ab = np.zeros((B, 65))
    Ab[:, 0:64] = np.exp(aM2D[:, 0:64] + aD2D[:, 1:65] - aM2D[:, 1:65])

    # layout: per row a fwd 66-col block and a bwd 66-col block; on device the
    # four groups' fwd blocks are contiguous (cols 0:264) then the bwd blocks
    def to_col(fwd65, bwd65_by_k):
        f = np.zeros((B, GS))
        f[:, 0:65] = fwd65
        bwd = np.zeros((B, GS))
        bwd[:, 0:65] = bwd65_by_k[:, ::-1]
        return f, bwd

    C1col = to_col(C1, C1)
    C2col = to_col(C2, C2)
    Qcol = to_col(Qf, Qb)
    Acol = to_col(Af, Ab)

    TABf = np.zeros((N, B, GS), np.float32)
    TABb = np.zeros((N, B, GS), np.float32)
    bidx = np.arange(B)[:, None]
    kf = np.arange(64)[None, :]
    kk = (64 - np.arange(1, 65))[None, :]
    for i in range(1, N + 1):
        TABf[i - 1, :, 1:65] = np.exp(
            aM2M[:, 1:65] + e[bidx, kf, x[:, i - 1][:, None]] - mu[:, None])
        TABb[i - 1, :, 1:65] = np.exp(
            aM2M[:, kk[0]] + e[bidx, kk, x[:, L - i][:, None]] - mu[:, None])

    fD0 = np.full((B, 65), -np.inf)
    fD0[:, 1] = aM2D[:, 0]
    fD0[:, 2:] = aM2D[:, 0:1] + np.cumsum(aD2D[:, 1:64], axis=1)
    gm0 = np.full((B, 65), -np.inf)
    gm0[:, 0] = aM2M[:, 0]
    gd0 = fD0 + aD2M
    gd0[:, 0] = -np.inf
    c0f = np.maximum(gm0.max(axis=1), gd0.max(axis=1))
    GM0 = np.exp(gm0 - c0f[:, None])
    GD0 = np.exp(gd0 - c0f[:, None])
    lnbD = np.empty((B, 65))
    lnbD[:, 64] = aD2M[:, 64]
    rev_cs = np.cumsum(aD2D[:, ::-1][:, 1:65], axis=1)[:, ::-1]
    lnbD[:, 0:64] = rev_cs + aD2M[:, 64:65]
    lnbM = np.empty((B, 65))
    lnbM[:, 64] = aM2M[:, 64]
    lnbM[:, 0:64] = aM2D[:, 0:64] + lnbD[:, 1:65]
    c0b = lnbM.max(axis=1)
    BP0 = np.exp(lnbM - c0b[:, None])
    SIa0 = np.zeros((B, 65))
    SIa0[:, 64] = np.exp(aM2I[:, 64] + aI2M[:, 64] - aI2I[:, 64] - c0b)

    P0 = (np.pad(GM0 + GD0, ((0, 0), (0, 1))),
          np.pad(BP0[:, ::-1], ((0, 0), (0, 1))))
    S0 = (np.pad(GM0, ((0, 0), (0, 1))), np.zeros((B, GS)))
    GIa0 = (np.zeros((B, GS)), np.pad(SIa0[:, ::-1], ((0, 0), (0, 1))))

    w1 = np.exp(-aM2M)
    w2 = np.exp(mu[:, None] - (aI2M + aM2I)) / 0.25

    def pack(fb):
        f, bwd = fb
        return np.concatenate(
            [_to_pg(f).reshape(NCORES, 128, F // 2),
             _to_pg(bwd).reshape(NCORES, 128, F // 2)], axis=2)

    c12 = np.concatenate([pack(C1col), pack(C2col)], axis=2)
    atil = pack(Acol)
    qcol = pack(Qcol)
    big0 = np.concatenate([pack(S0), pack(GIa0)], axis=2)
    p0 = pack((P0[0], P0[1]))
    p0 = np.concatenate([np.zeros((NCORES, 128, 2)), p0], axis=2)
    # per-partition CACC init: groups of one partition share the scale terms
    # only through the host finish, which reads per-row c0 separately; device
    # CACC tracks only the (shared) rescale exponents, init 0.
    cinit = np.zeros((NCORES, 128, 8), np.float32)
    tab = np.concatenate([
        TABf.reshape(N, NCORES, G, 128, GS).transpose(1, 0, 3, 2, 4)
            .reshape(NCORES, N, 128, F // 2),
        TABb.reshape(N, NCORES, G, 128, GS).transpose(1, 0, 3, 2, 4)
            .reshape(NCORES, N, 128, F // 2)], axis=3)

    in_maps = []
    for c in range(NCORES):
        in_maps.append({
            "tab": np.ascontiguousarray(tab[c]).astype(BF16),
            "c12": np.ascontiguousarray(c12[c]).astype(BF16),
            "aq": np.ascontiguousarray(
                np.concatenate([atil[c], qcol[c]], axis=1)).astype(BF16),
            "big0": np.ascontiguousarray(big0[c]).astype(BF16),
            "p0": np.ascontiguousarray(p0[c]).astype(BF16),
            "cinit": np.ascontiguousarray(cinit[c]),
        })
    host = dict(w1=w1, w2=w2, C1=C1, C2=C2, mu=mu,
                c0=np.stack([c0f, c0b], axis=1))
    return in_maps, host


def _host_finish(res, host, mus, logvars):
    w1, w2 = host["w1"], host["w2"]
    C1, C2, mu, c0 = host["C1"], host["C2"], host["mu"], host["c0"]

    SG = np.stack([np.asarray(res.results[c]["sgout"]).astype(np.float32)
                   for c in range(NCORES)])
    PO = np.stack([np.asarray(res.results[c]["pout"]).astype(np.float32)
                   for c in range(NCORES)])
    CA = np.stack([np.asarray(res.results[c]["cacc"], np.float32)
                   for c in range(NCORES)])                 # [NC,128,2]

    H = F // 2

    def rows(arr, lo):  # [NC,128,F-ish] half-slice -> [B,GS]
        return arr[:, :, lo:lo + H].reshape(NCORES, 128, G, GS) \
            .transpose(0, 2, 1, 3).reshape(B, GS)

    Pf = rows(PO, 0).astype(np.float64)
    Pb = rows(PO, H).astype(np.float64)
    Sf = rows(SG, 0).astype(np.float64)
    Sb = rows(SG, H).astype(np.float64)
    Gf = rows(SG[:, :, F:2 * F], 0).astype(np.float64)
    Gb = rows(SG[:, :, F:2 * F], H).astype(np.float64)
    # device CACC: [NC,128,8] = fwd g0..g3, bwd g0..g3 per partition
    cf_rows = CA[:, :, 0:4].transpose(0, 2, 1).reshape(B)
    cb_rows = CA[:, :, 4:8].transpose(0, 2, 1).reshape(B)

    GM = Sf[:, 0:65]
    GI = Gf[:, 0:65]
    bM = Pb[:, 0:65][:, ::-1]
    SIx = (C1 * Sb[:, 0:65][:, ::-1] + C2 * Gb[:, 0:65][:, ::-1])
    tot = (GM * bM * w1 + GI * SIx * w2).sum(axis=1)
    lnP = np.log(np.maximum(tot, 1e-300)) + c0[:, 0] + c0[:, 1] \
        + cf_rows + cb_rows + L * mu
    recon = float(np.mean(-lnP))

    mus = np.asarray(mus, np.float64)
    lv = np.asarray(logvars, np.float64)
    kld = float(np.mean(-0.5 * np.sum(1.0 + lv - mus * mus - np.exp(lv),
                                      axis=1)))
    return np.float32(recon + kld)


def kernel(batch_input, transition_probs, emission_probs, mus, logvars,
           _trace=False, _trace_kwargs=None):
    from concourse.bass_utils import run_bass_kernel_spmd

    if "nc" not in _CACHE:
        _CACHE["nc"] = _build_program()
    nc = _CACHE["nc"]

    in_maps, host = _host_prep(batch_input, transition_probs, emission_probs)
    kw = {}
    if _trace:
        kw["trace"] = True
        kw.update(_trace_kwargs or {})
    res = run_bass_kernel_spmd(nc, in_maps, list(range(NCORES)), **kw)
    _CACHE["last_results"] = res

    return _host_finish(res, host, mus, logvars)
